# revision 2
# baseline (speedup 1.0000x reference)
"""Trainium2 Bass kernel for the BreakthroughSNN encoder problem — v2.

Computation per (b, t, s, d):
    out = w0*rate + w1*temporal + w2*pop + w3*phase   (w = softmax(enc_weights))

v2 engine assignment (vs fp32 baseline):
  - all bulk tensors 16-bit: pop_rand f16, rate_rand u16, W/embT f16 matmul,
    output bf16 (exact: values are k/8 with k<=33) with host affine 0.25*(v-1)
  - DVE per t: pop compare (f16 2x), 3-level halving tree (f16 2x),
    rate compare (u16 2x), phase/temporal/pop folds (STT f16 2x)
  - ACT per t: temporal one-hot via Square+Relu (1[st==t] = relu(1-(st-t)^2)),
    Sin for the phase waves
  - PE: pop matmul in f16, wave args accumulated in PSUM via identity/ones
    matmuls with f16 hi/lo splits for precision
Sharding: (b,s) token axis split 8 ways (128 tokens/core); pop_W replicated.
"""

import os
import sys

for _p in ("/opt/trn_rl_repo", os.path.expanduser("~/.axon_site/_ro/trn_rl_repo")):
    if os.path.isdir(_p) and _p not in sys.path:
        sys.path.insert(0, _p)

import numpy as np

import concourse.bacc as bacc
import concourse.mybir as mybir
import concourse.tile as tile
from concourse.bass_utils import run_bass_kernel_spmd

Alu = mybir.AluOpType
Act = mybir.ActivationFunctionType
F32 = mybir.dt.float32
F16 = mybir.dt.float16
BF16 = mybir.dt.bfloat16
U16 = mybir.dt.uint16

TWO_PI = 2.0 * np.pi

B, T, S, D, N = 4, 16, 256, 512, 8
NCORES = 8
NTOK = B * S                 # 1024 tokens
TOK = NTOK // NCORES         # 128 tokens per core
DN = D * N                   # 4096
HF = DN // 2                 # 2048
TC = 4                       # t-steps per wave chunk
CH = TC * D                  # 2048 columns per wave chunk

# HW-measured: ACT Sin is only accurate on ~[-3.3, 3.3]; the (-2pi, 2pi)
# extended-range variant is wrong (err up to 5.0).  Keep the relu/sign fold.
USE_EXT_SIN = False


def _build_program(w0, w1, w2, w3, has_bias):
    from contextlib import ExitStack

    nk = D // 128 + (1 if has_bias else 0)
    kdim = nk * 128
    uniform = abs(w1 - w0) < 1e-12 and abs(w3 - w0) < 1e-12 and abs(w2 - w0) < 1e-12
    assert uniform, "non-uniform enc_weights not supported by v2 (falls back)"

    nc = bacc.Bacc("TRN2", target_bir_lowering=False, debug=False,
                   num_devices=NCORES)

    emb = nc.dram_tensor("emb", [TOK, D], F32, kind="ExternalInput")
    embT = nc.dram_tensor("embT", [kdim, TOK], BF16, kind="ExternalInput")
    noise = nc.dram_tensor("noise", [TOK, D], F32, kind="ExternalInput")
    rr = nc.dram_tensor("rr", [T, TOK, D], U16, kind="ExternalInput")
    pr = nc.dram_tensor("pr", [T, TOK, DN], F16, kind="ExternalInput")
    Wd = nc.dram_tensor("W", [kdim, DN], BF16, kind="ExternalInput")
    tfd = nc.dram_tensor("tf", [2, T * D], BF16, kind="ExternalInput")
    identd = nc.dram_tensor("ident", [128, 128], BF16, kind="ExternalInput")
    diagd = nc.dram_tensor("diag2", [128, 256], F16, kind="ExternalInput")
    outd = nc.dram_tensor("out", [T, TOK, D], mybir.dt.uint8, kind="ExternalOutput")

    with tile.TileContext(nc) as tc, ExitStack() as ctx, \
            nc.allow_low_precision(reason="16-bit lanes carry exact small ints"):
        const = ctx.enter_context(tc.tile_pool(name="const", bufs=1))
        wp = ctx.enter_context(tc.tile_pool(name="wp", bufs=2))
        psum = ctx.enter_context(tc.tile_pool(name="psum", bufs=2, space="PSUM"))
        lp = ctx.enter_context(tc.tile_pool(name="lp", bufs=2))
        sp = ctx.enter_context(tc.tile_pool(name="sp", bufs=1))

        # ---- one-time loads ----
        ident = const.tile([128, 128], BF16)
        nc.sync.dma_start(ident[:], identd[:])
        diag2 = const.tile([128, 256], F16, tag="diag2")
        nc.sync.dma_start(diag2[:], diagd[:])
        ones_row = const.tile([1, 128], BF16)
        nc.vector.memset(ones_row[:], 1.0)
        emb_sb = const.tile([TOK, D], F32)
        nc.sync.dma_start(emb_sb[:], emb[:])
        noise_sb = const.tile([TOK, D], F32)
        nc.sync.dma_start(noise_sb[:], noise[:])
        tfh = const.tile([1, T * D], BF16, tag="tfh")
        nc.sync.dma_start(tfh[:], tfd[0:1, :])
        tfl = const.tile([1, T * D], BF16, tag="tfl")
        nc.sync.dma_start(tfl[:], tfd[1:2, :])
        lhsT = []
        for k in range(nk):
            lt = const.tile([128, TOK], BF16, tag=f"lhsT{k}")
            nc.sync.dma_start(lt[:], embT[k * 128:(k + 1) * 128, :])
            lhsT.append(lt)
        # per-t bias columns (-t) for the temporal Square trick
        tbias = []
        for t in range(T):
            bt = const.tile([128, 1], F32, tag=f"tb{t}")
            nc.vector.memset(bt[:], -float(t))
            tbias.append(bt)

        # ---- per-token precompute ----
        sig = const.tile([TOK, D], F32)
        nc.scalar.activation(sig[:], emb_sb[:], Act.Sigmoid)

        # rates -> u16 threshold on the 65535 grid
        tmp = const.tile([TOK, D], F32)
        nc.vector.tensor_scalar(tmp[:], sig[:], 0.9, 0.05, Alu.mult, Alu.add)
        nc.vector.scalar_tensor_tensor(tmp[:], noise_sb[:], 0.1, tmp[:],
                                       Alu.mult, Alu.add)
        rates = const.tile([TOK, D], F32)
        nc.vector.tensor_scalar(rates[:], tmp[:], 0.0, 1.0, Alu.max, Alu.min)
        nc.vector.tensor_scalar(tmp[:], rates[:], 65535.0, None, Alu.mult)
        rth = const.tile([TOK, D], U16)
        nc.vector.tensor_copy(rth[:], tmp[:])

        # st = floor(sig*(T-1)) as f16 ints
        x15 = const.tile([TOK, D], F32)
        nc.vector.tensor_scalar(x15[:], sig[:], float(T - 1), None, Alu.mult)
        rnd = const.tile([TOK, D], F32)
        nc.vector.tensor_scalar(rnd[:], x15[:], 8388608.0, 8388608.0,
                                Alu.add, Alu.subtract)
        gtt = const.tile([TOK, D], F32)
        nc.vector.tensor_tensor(gtt[:], rnd[:], x15[:], Alu.is_gt)
        st = const.tile([TOK, D], F16)
        nc.vector.tensor_tensor(st[:], rnd[:], gtt[:], Alu.subtract)

        # phases hi/lo f16 split (phi = sig*2pi)
        phi = const.tile([TOK, D], F32)
        nc.vector.tensor_scalar(phi[:], sig[:], TWO_PI, None, Alu.mult)
        phih = const.tile([TOK, D], BF16)
        nc.vector.tensor_copy(phih[:], phi[:])
        phil = const.tile([TOK, D], BF16)
        nc.vector.tensor_tensor(phil[:], phi[:], phih[:], Alu.subtract)

        # ---- pop linear: thr = sigmoid(embT.T @ W) in f16, n-major cols ----
        thr = const.tile([TOK, DN], F16)
        for h in range(2):
            ps = psum.tile([128, HF], F32, tag="pp")
            for k in range(nk):
                wt = wp.tile([128, HF], BF16, tag="w")
                nc.sync.dma_start(wt[:], Wd[k * 128:(k + 1) * 128,
                                            h * HF:(h + 1) * HF])
                for j in range(HF // 512):
                    sl = slice(j * 512, (j + 1) * 512)
                    nc.tensor.matmul(ps[:, sl], lhsT[k][:], wt[:, sl],
                                     start=(k == 0), stop=(k == nk - 1))
            nc.scalar.activation(thr[:, h * HF:(h + 1) * HF], ps[:], Act.Sigmoid)

        # ---- wave chunks ----
        # USE_EXT_SIN: psw = phi + ((t*f mod 2pi centered) - pi) in (-2pi,2pi),
        #   wv = sin(psw); wave = -wv, so phase bit = 1[wv < -0.5].
        # else: psw = phi + (t*f mod 2pi centered) in (-pi, 3pi); fold via
        #   relu/sign to (-pi, pi], wv = sin; phase bit = 1[wv > 0.5].
        if not USE_EXT_SIN:
            neg_pi = const.tile([128, 1], F32, tag="negpi")
            nc.vector.memset(neg_pi[:], -float(np.float32(np.pi)))

        def emit_waves_chunk(c):
            psw = psum.tile([128, CH], F32, name=f"psw{c}", tag="pp")
            for j in range(TC):
                sl = slice(j * 512, (j + 1) * 512)
                nc.tensor.matmul(psw[:, sl], ident[:], phih[:], start=True,
                                 stop=False)
            for j in range(TC):
                sl = slice(j * 512, (j + 1) * 512)
                nc.tensor.matmul(psw[:, sl], ident[:], phil[:], start=False,
                                 stop=False)
            for rowt, last in ((tfh, False), (tfl, True)):
                for j in range(TC):
                    sl = slice(j * 512, (j + 1) * 512)
                    nc.tensor.matmul(psw[:, sl], ones_row[:],
                                     rowt[0:1, c * CH + j * 512:
                                          c * CH + (j + 1) * 512],
                                     start=False, stop=last)
            # fold (-pi,3pi)->(-pi,pi]: s = 1[arg>=pi] on ACT, then
            # psw += -2pi*s via two accumulate-after-stop diag matmuls
            fold = sp.tile([TOK, CH], F16, tag="fold")
            nc.scalar.activation(fold[:], psw[:], Act.Relu, bias=neg_pi[:])
            nc.scalar.activation(fold[:], fold[:], Act.Sign)
            for dcol, last in ((0, False), (1, True)):
                for j in range(TC):
                    sl = slice(j * 512, (j + 1) * 512)
                    nc.tensor.matmul(psw[:, sl],
                                     diag2[:, dcol * 128:(dcol + 1) * 128],
                                     fold[:, sl], start=False, stop=last,
                                     skip_group_check=True)
            wv = lp.tile([TOK, CH], F16, tag="wv")
            nc.scalar.activation(wv[:], psw[:], Act.Sin)
            return wv

        # rth replicated 4x for the batched rate compare
        rth4 = const.tile([TOK, TC * D], U16, tag="rth4")
        for j in range(TC):
            nc.vector.tensor_copy(rth4[:, j * D:(j + 1) * D], rth[:])

        NCH = T // TC
        # ---- prologue hoist: rate compares + temporal one-hots for all
        # chunks run while the pop matmul/thr is still streaming on PE ----
        s2s = []
        for c in range(NCH):
            rr4 = lp.tile([TOK, TC * D], U16, tag="rr4")
            for j in range(TC):
                nc.sync.dma_start(rr4[:, j * D:(j + 1) * D], rr[c * TC + j])
            temp4 = lp.tile([TOK, TC * D], F16, tag="temp4")
            for j in range(TC):
                t = c * TC + j
                sq = sp.tile([TOK, D], F16, tag="sq")
                nc.scalar.activation(sq[:], st[:], Act.Square,
                                     bias=tbias[t][:], scale=1.0)
                nc.scalar.activation(temp4[:, j * D:(j + 1) * D], sq[:],
                                     Act.Relu, bias=1.0, scale=-1.0)
            o1 = sp.tile([TOK, TC * D], F16, tag="o1")
            nc.vector.tensor_tensor(o1[:], rr4[:], rth4[:], Alu.is_lt)
            s2 = lp.tile([TOK, TC * D], F16, tag=f"s2_{c}")
            nc.vector.tensor_tensor(s2[:], o1[:], temp4[:], Alu.add)
            s2s.append(s2)

        # ---- t-loop backbone: pop compares/tree in 2-t passes; waves and
        # the remaining combine interleave per chunk ----
        for t in range(T):
            tl = t % TC
            c = t // TC
            if tl == 0:
                wv4 = emit_waves_chunk(c)
                cnt4 = lp.tile([TOK, TC * D], F16, tag="cnt4")

            if tl % 2 == 0:
                pt2 = lp.tile([TOK, 2 * DN], F16, tag="pt2")
                nc.sync.dma_start(pt2[:, 0:DN], pr[t])
                nc.sync.dma_start(pt2[:, DN:2 * DN], pr[t + 1])
                nc.vector.tensor_tensor(pt2[:, 0:DN], pt2[:, 0:DN], thr[:],
                                        Alu.is_lt)
                nc.vector.tensor_tensor(pt2[:, DN:2 * DN], pt2[:, DN:2 * DN],
                                        thr[:], Alu.is_lt)
                spk3 = pt2[:].rearrange("p (t x) -> p t x", t=2)
                h1 = sp.tile([TOK, 2 * HF], F16, tag="h1")
                h13 = h1[:].rearrange("p (t x) -> p t x", t=2)
                nc.vector.tensor_tensor(h13, spk3[:, :, 0:HF],
                                        spk3[:, :, HF:DN], Alu.add)
                h2 = sp.tile([TOK, HF], F16, tag="h2")
                h23 = h2[:].rearrange("p (t x) -> p t x", t=2)
                nc.vector.tensor_tensor(h23, h13[:, :, 0:HF // 2],
                                        h13[:, :, HF // 2:HF], Alu.add)
                nc.vector.tensor_tensor(
                    cnt4[:, tl * D:(tl + 2) * D].rearrange(
                        "p (t x) -> p t x", t=2),
                    h23[:, :, 0:D], h23[:, :, D:2 * D], Alu.add)

            if tl == TC - 1:
                # finish combine: s2 += phase, then out_u8 = 8*s2 + cnt
                ph = sp.tile([TOK, TC * D], F16, tag="ph")
                nc.vector.tensor_scalar(ph[:], wv4[:], 0.5, None, Alu.is_gt)
                s2 = s2s[c]
                nc.vector.tensor_tensor(s2[:], s2[:], ph[:], Alu.add)
                outst = lp.tile([TOK, TC * D], mybir.dt.uint8, tag="outst")
                nc.vector.scalar_tensor_tensor(outst[:], s2[:], 8.0,
                                               cnt4[:], Alu.mult, Alu.add)
                for j in range(TC):
                    nc.sync.dma_start(outd[t - TC + 1 + j],
                                      outst[:, j * D:(j + 1) * D])

    nc.compile()
    return nc


def _prepare_inputs(embeddings, pop_W, pop_b, freq_bands, enc_weights,
                    rate_noise, rate_rand, pop_rand):
    e = np.exp(enc_weights.astype(np.float64)
               - enc_weights.astype(np.float64).max())
    w = (e / e.sum()).astype(np.float32)
    w0, w1, w2, w3 = [float(x) for x in w]

    has_bias = bool(np.any(pop_b != 0))
    kdim = D + (128 if has_bias else 0)

    emb_f = np.ascontiguousarray(embeddings.reshape(NTOK, D))
    noise_f = np.ascontiguousarray(rate_noise.reshape(NTOK, D))
    # rate_rand [B,T,S,D] -> [BS, T, D] quantized to the 65535 grid
    rr_q = np.minimum(rate_rand.astype(np.float64) * 65535.0,
                      65535.0).astype(np.uint16)
    rr_f = np.ascontiguousarray(rr_q.transpose(0, 2, 1, 3).reshape(NTOK, T, D))
    # pop_rand [B,T,S,D,N] -> [BS, T, N*D] (n-major) in f16
    pr_f = np.ascontiguousarray(
        pop_rand.astype(np.float16).transpose(0, 2, 1, 4, 3)
        .reshape(NTOK, T, DN))
    # pop_W columns to n-major, f16
    W2 = np.ascontiguousarray(pop_W.reshape(D, D, N).transpose(0, 2, 1)
                              .reshape(D, DN))
    if has_bias:
        b_nm = np.ascontiguousarray(pop_b.reshape(D, N).T.reshape(1, DN))
        W2 = np.vstack([W2, b_nm, np.zeros((127, DN), np.float32)])
    import ml_dtypes
    W2 = np.ascontiguousarray(W2.astype(ml_dtypes.bfloat16))

    # wave rows: m[t,d] = (f32(t*f) mod 2pi centered to (-pi,pi]) - pi,
    # f16 hi/lo split.  matches jnp.linspace bit-exactly.
    import jax
    import jax.numpy as jnp
    with jax.default_device(jax.devices("cpu")[0]):
        t_lin = np.asarray(jnp.linspace(0.0, TWO_PI, T)).astype(np.float64)
    tfc = (t_lin[:, None] * freq_bands.astype(np.float64)[None, :]
           ).astype(np.float32).astype(np.float64)      # f32(t*f) as jax makes
    k0 = np.round(tfc / (2.0 * np.pi))
    m = tfc - k0 * (2.0 * np.pi)                        # in (-pi, pi]
    if USE_EXT_SIN:
        m = m - np.pi                                   # in (-2pi, 0]
    import ml_dtypes
    m_hi = m.astype(ml_dtypes.bfloat16)
    m_lo = (m - m_hi.astype(np.float64)).astype(ml_dtypes.bfloat16)
    tf = np.stack([m_hi.reshape(-1), m_lo.reshape(-1)])
    ident = np.eye(128, dtype=ml_dtypes.bfloat16)
    c_hi = 6.28125
    c_lo = 2.0 * np.pi - c_hi
    diag2 = np.hstack([np.eye(128) * (-c_hi),
                       np.eye(128) * (-c_lo)]).astype(np.float16)

    import ml_dtypes
    in_maps = []
    for c in range(NCORES):
        t0, t1 = c * TOK, (c + 1) * TOK
        embT = emb_f[t0:t1].T
        if has_bias:
            embT = np.vstack([embT, np.ones((1, TOK), np.float32),
                              np.zeros((127, TOK), np.float32)])
        in_maps.append({
            "emb": emb_f[t0:t1],
            "embT": np.ascontiguousarray(embT.astype(ml_dtypes.bfloat16)),
            "noise": noise_f[t0:t1],
            "rr": np.ascontiguousarray(rr_f[t0:t1].transpose(1, 0, 2)),
            "pr": np.ascontiguousarray(pr_f[t0:t1].transpose(1, 0, 2)),
            "W": W2,
            "tf": tf,
            "ident": ident,
            "diag2": diag2,
        })
    return in_maps, (w0, w1, w2, w3), has_bias


_cache = {}


def kernel(embeddings, pop_W, pop_b, freq_bands, enc_weights,
           rate_noise, rate_rand, pop_rand, _want_trace=False):
    in_maps, (w0, w1, w2, w3), has_bias = _prepare_inputs(
        embeddings, pop_W, pop_b, freq_bands, enc_weights,
        rate_noise, rate_rand, pop_rand)

    key = (w0, w1, w2, w3, has_bias)
    if key not in _cache:
        _cache[key] = _build_program(w0, w1, w2, w3, has_bias)
    nc = _cache[key]

    res = run_bass_kernel_spmd(nc, in_maps, core_ids=list(range(NCORES)),
                               trace=_want_trace)

    # out per core: [T, TOK, D] u8 holding 8*(r+temp+ph)+cnt -> f32 v/32
    full = np.empty((NTOK, T, D), np.float32)
    for c in range(NCORES):
        v = np.asarray(res.results[c]["out"]).astype(np.float32)
        full[c * TOK:(c + 1) * TOK] = v.transpose(1, 0, 2)
    out = full * (1.0 / 32.0)
    out = out.reshape(B, S, T, D).transpose(0, 2, 1, 3)
    out = np.ascontiguousarray(out.astype(np.float32))
    if _want_trace:
        kernel._last_trace = res
    return out


# revision 3
# speedup vs baseline: 1.0100x; 1.0100x over previous
"""Trainium2 Bass kernel for the BreakthroughSNN encoder problem — v2.

Computation per (b, t, s, d):
    out = w0*rate + w1*temporal + w2*pop + w3*phase   (w = softmax(enc_weights))

v2 engine assignment (vs fp32 baseline):
  - all bulk tensors 16-bit: pop_rand f16, rate_rand u16, W/embT f16 matmul,
    output bf16 (exact: values are k/8 with k<=33) with host affine 0.25*(v-1)
  - DVE per t: pop compare (f16 2x), 3-level halving tree (f16 2x),
    rate compare (u16 2x), phase/temporal/pop folds (STT f16 2x)
  - ACT per t: temporal one-hot via Square+Relu (1[st==t] = relu(1-(st-t)^2)),
    Sin for the phase waves
  - PE: pop matmul in f16, wave args accumulated in PSUM via identity/ones
    matmuls with f16 hi/lo splits for precision
Sharding: (b,s) token axis split 8 ways (128 tokens/core); pop_W replicated.
"""

import os
import sys

for _p in ("/opt/trn_rl_repo", os.path.expanduser("~/.axon_site/_ro/trn_rl_repo")):
    if os.path.isdir(_p) and _p not in sys.path:
        sys.path.insert(0, _p)

import numpy as np

import concourse.bacc as bacc
import concourse.mybir as mybir
import concourse.tile as tile
from concourse.bass_utils import run_bass_kernel_spmd

Alu = mybir.AluOpType
Act = mybir.ActivationFunctionType
F32 = mybir.dt.float32
F16 = mybir.dt.float16
BF16 = mybir.dt.bfloat16
U16 = mybir.dt.uint16

TWO_PI = 2.0 * np.pi

B, T, S, D, N = 4, 16, 256, 512, 8
NCORES = 8
NTOK = B * S                 # 1024 tokens
TOK = NTOK // NCORES         # 128 tokens per core
DN = D * N                   # 4096
HF = DN // 2                 # 2048
TC = 4                       # t-steps per wave chunk
CH = TC * D                  # 2048 columns per wave chunk

# HW-measured: ACT Sin is only accurate on ~[-3.3, 3.3]; the (-2pi, 2pi)
# extended-range variant is wrong (err up to 5.0).  Keep the relu/sign fold.
USE_EXT_SIN = False


def _build_program(w0, w1, w2, w3, has_bias):
    from contextlib import ExitStack

    nk = D // 128 + (1 if has_bias else 0)
    kdim = nk * 128
    uniform = abs(w1 - w0) < 1e-12 and abs(w3 - w0) < 1e-12 and abs(w2 - w0) < 1e-12
    assert uniform, "non-uniform enc_weights not supported by v2 (falls back)"

    nc = bacc.Bacc("TRN2", target_bir_lowering=False, debug=False,
                   num_devices=NCORES)

    emb = nc.dram_tensor("emb", [TOK, D], F32, kind="ExternalInput")
    embT = nc.dram_tensor("embT", [kdim, TOK], BF16, kind="ExternalInput")
    noise = nc.dram_tensor("noise", [TOK, D], F32, kind="ExternalInput")
    rr = nc.dram_tensor("rr", [T, TOK, D], U16, kind="ExternalInput")
    pr = nc.dram_tensor("pr", [T, TOK, DN], F16, kind="ExternalInput")
    Wd = nc.dram_tensor("W", [kdim, DN], BF16, kind="ExternalInput")
    tfd = nc.dram_tensor("tf", [2, T * D], BF16, kind="ExternalInput")
    identd = nc.dram_tensor("ident", [128, 128], BF16, kind="ExternalInput")
    diagd = nc.dram_tensor("diag2", [128, 256], F16, kind="ExternalInput")
    outd = nc.dram_tensor("out", [T, TOK, D], mybir.dt.uint8, kind="ExternalOutput")

    with tile.TileContext(nc) as tc, ExitStack() as ctx, \
            nc.allow_low_precision(reason="16-bit lanes carry exact small ints"):
        const = ctx.enter_context(tc.tile_pool(name="const", bufs=1))
        wp = ctx.enter_context(tc.tile_pool(name="wp", bufs=2))
        psum = ctx.enter_context(tc.tile_pool(name="psum", bufs=2, space="PSUM"))
        lp = ctx.enter_context(tc.tile_pool(name="lp", bufs=2))
        sp = ctx.enter_context(tc.tile_pool(name="sp", bufs=1))

        # ---- one-time loads ----
        ident = const.tile([128, 128], BF16)
        nc.sync.dma_start(ident[:], identd[:])
        diag2 = const.tile([128, 256], F16, tag="diag2")
        nc.sync.dma_start(diag2[:], diagd[:])
        ones_row = const.tile([1, 128], BF16)
        nc.vector.memset(ones_row[:], 1.0)
        emb_sb = const.tile([TOK, D], F32)
        nc.sync.dma_start(emb_sb[:], emb[:])
        noise_sb = const.tile([TOK, D], F32)
        nc.sync.dma_start(noise_sb[:], noise[:])
        tfh = const.tile([1, T * D], BF16, tag="tfh")
        nc.sync.dma_start(tfh[:], tfd[0:1, :])
        tfl = const.tile([1, T * D], BF16, tag="tfl")
        nc.sync.dma_start(tfl[:], tfd[1:2, :])
        lhsT = []
        for k in range(nk):
            lt = const.tile([128, TOK], BF16, tag=f"lhsT{k}")
            nc.sync.dma_start(lt[:], embT[k * 128:(k + 1) * 128, :])
            lhsT.append(lt)
        # per-t bias columns (-t) for the temporal Square trick
        tbias = []
        for t in range(T):
            bt = const.tile([128, 1], F32, tag=f"tb{t}")
            nc.vector.memset(bt[:], -float(t))
            tbias.append(bt)

        # ---- per-token precompute ----
        sig = const.tile([TOK, D], F32)
        nc.scalar.activation(sig[:], emb_sb[:], Act.Sigmoid)

        # rates -> u16 threshold on the 65535 grid
        tmp = const.tile([TOK, D], F32)
        nc.vector.tensor_scalar(tmp[:], sig[:], 0.9, 0.05, Alu.mult, Alu.add)
        nc.vector.scalar_tensor_tensor(tmp[:], noise_sb[:], 0.1, tmp[:],
                                       Alu.mult, Alu.add)
        rates = const.tile([TOK, D], F32)
        nc.vector.tensor_scalar(rates[:], tmp[:], 0.0, 1.0, Alu.max, Alu.min)
        nc.vector.tensor_scalar(tmp[:], rates[:], 65535.0, None, Alu.mult)
        rth = const.tile([TOK, D], U16)
        nc.vector.tensor_copy(rth[:], tmp[:])

        # st = floor(sig*(T-1)) as f16 ints
        x15 = const.tile([TOK, D], F32)
        nc.vector.tensor_scalar(x15[:], sig[:], float(T - 1), None, Alu.mult)
        rnd = const.tile([TOK, D], F32)
        nc.vector.tensor_scalar(rnd[:], x15[:], 8388608.0, 8388608.0,
                                Alu.add, Alu.subtract)
        gtt = const.tile([TOK, D], F32)
        nc.vector.tensor_tensor(gtt[:], rnd[:], x15[:], Alu.is_gt)
        st = const.tile([TOK, D], F16)
        nc.vector.tensor_tensor(st[:], rnd[:], gtt[:], Alu.subtract)

        # phases hi/lo f16 split (phi = sig*2pi)
        phi = const.tile([TOK, D], F32)
        nc.vector.tensor_scalar(phi[:], sig[:], TWO_PI, None, Alu.mult)
        phih = const.tile([TOK, D], BF16)
        nc.vector.tensor_copy(phih[:], phi[:])
        phil = const.tile([TOK, D], BF16)
        nc.vector.tensor_tensor(phil[:], phi[:], phih[:], Alu.subtract)

        # ---- pop linear: thr = sigmoid(embT.T @ W) in f16, n-major cols ----
        thr = const.tile([TOK, DN], F16)
        for h in range(2):
            ps = psum.tile([128, HF], F32, tag="pp")
            for k in range(nk):
                wt = wp.tile([128, HF], BF16, tag="w")
                nc.sync.dma_start(wt[:], Wd[k * 128:(k + 1) * 128,
                                            h * HF:(h + 1) * HF])
                for j in range(HF // 512):
                    sl = slice(j * 512, (j + 1) * 512)
                    nc.tensor.matmul(ps[:, sl], lhsT[k][:], wt[:, sl],
                                     start=(k == 0), stop=(k == nk - 1))
            nc.scalar.activation(thr[:, h * HF:(h + 1) * HF], ps[:], Act.Sigmoid)

        # ---- wave chunks ----
        # USE_EXT_SIN: psw = phi + ((t*f mod 2pi centered) - pi) in (-2pi,2pi),
        #   wv = sin(psw); wave = -wv, so phase bit = 1[wv < -0.5].
        # else: psw = phi + (t*f mod 2pi centered) in (-pi, 3pi); fold via
        #   relu/sign to (-pi, pi], wv = sin; phase bit = 1[wv > 0.5].
        if not USE_EXT_SIN:
            neg_pi = const.tile([128, 1], F32, tag="negpi")
            nc.vector.memset(neg_pi[:], -float(np.float32(np.pi)))

        def emit_waves_chunk(c):
            psw = psum.tile([128, CH], F32, name=f"psw{c}", tag="pp")
            for j in range(TC):
                sl = slice(j * 512, (j + 1) * 512)
                nc.tensor.matmul(psw[:, sl], ident[:], phih[:], start=True,
                                 stop=False)
            for j in range(TC):
                sl = slice(j * 512, (j + 1) * 512)
                nc.tensor.matmul(psw[:, sl], ident[:], phil[:], start=False,
                                 stop=False)
            for rowt, last in ((tfh, False), (tfl, True)):
                for j in range(TC):
                    sl = slice(j * 512, (j + 1) * 512)
                    nc.tensor.matmul(psw[:, sl], ones_row[:],
                                     rowt[0:1, c * CH + j * 512:
                                          c * CH + (j + 1) * 512],
                                     start=False, stop=last)
            # fold (-pi,3pi)->(-pi,pi]: s = 1[arg>=pi] on ACT, then
            # psw += -2pi*s via two accumulate-after-stop diag matmuls
            fold = sp.tile([TOK, CH], F16, tag="fold")
            nc.scalar.activation(fold[:], psw[:], Act.Relu, bias=neg_pi[:])
            nc.scalar.activation(fold[:], fold[:], Act.Sign)
            for dcol, last in ((0, False), (1, True)):
                for j in range(TC):
                    sl = slice(j * 512, (j + 1) * 512)
                    nc.tensor.matmul(psw[:, sl],
                                     diag2[:, dcol * 128:(dcol + 1) * 128],
                                     fold[:, sl], start=False, stop=last,
                                     skip_group_check=True)
            wv = lp.tile([TOK, CH], F16, tag="wv")
            nc.scalar.activation(wv[:], psw[:], Act.Sin)
            return wv

        # rth replicated 4x for the batched rate compare
        rth4 = const.tile([TOK, TC * D], U16, tag="rth4")
        for j in range(TC):
            nc.vector.tensor_copy(rth4[:, j * D:(j + 1) * D], rth[:])

        NCH = T // TC
        # ---- prologue hoist: rate compares + temporal one-hots for all
        # chunks run while the pop matmul/thr is still streaming on PE ----
        s2s = []
        for c in range(NCH):
            rr4 = lp.tile([TOK, TC * D], U16, tag="rr4")
            for j in range(TC):
                nc.sync.dma_start(rr4[:, j * D:(j + 1) * D], rr[c * TC + j])
            temp4 = lp.tile([TOK, TC * D], F16, tag="temp4")
            for j in range(TC):
                t = c * TC + j
                sq = sp.tile([TOK, D], F16, tag="sq")
                nc.scalar.activation(sq[:], st[:], Act.Square,
                                     bias=tbias[t][:], scale=1.0)
                nc.scalar.activation(temp4[:, j * D:(j + 1) * D], sq[:],
                                     Act.Relu, bias=1.0, scale=-1.0)
            o1 = sp.tile([TOK, TC * D], F16, tag="o1")
            nc.vector.tensor_tensor(o1[:], rr4[:], rth4[:], Alu.is_lt)
            s2 = lp.tile([TOK, TC * D], F16, tag=f"s2_{c}")
            nc.vector.tensor_tensor(s2[:], o1[:], temp4[:], Alu.add)
            s2s.append(s2)

        # ---- t-loop backbone: pop compares/tree in 2-t passes; waves and
        # the remaining combine interleave per chunk ----
        for t in range(T):
            tl = t % TC
            c = t // TC
            if tl == 0:
                wv4 = emit_waves_chunk(c)
                cnt4 = lp.tile([TOK, TC * D], F16, tag="cnt4")

            if tl % 2 == 0:
                pt2 = lp.tile([TOK, 2 * DN], F16, tag="pt2")
                nc.sync.dma_start(pt2[:, 0:DN], pr[t])
                nc.sync.dma_start(pt2[:, DN:2 * DN], pr[t + 1])
                if t == 0:
                    # first pair: compare in thr-halves so the DVE backbone
                    # starts as soon as the first matmul half lands
                    for base in (0, DN):
                        nc.vector.tensor_tensor(
                            pt2[:, base:base + HF], pt2[:, base:base + HF],
                            thr[:, 0:HF], Alu.is_lt)
                        nc.vector.tensor_tensor(
                            pt2[:, base + HF:base + DN],
                            pt2[:, base + HF:base + DN],
                            thr[:, HF:DN], Alu.is_lt)
                else:
                    nc.vector.tensor_tensor(pt2[:, 0:DN], pt2[:, 0:DN],
                                            thr[:], Alu.is_lt)
                    nc.vector.tensor_tensor(pt2[:, DN:2 * DN],
                                            pt2[:, DN:2 * DN],
                                            thr[:], Alu.is_lt)
                spk3 = pt2[:].rearrange("p (t x) -> p t x", t=2)
                h1 = sp.tile([TOK, 2 * HF], F16, tag="h1")
                h13 = h1[:].rearrange("p (t x) -> p t x", t=2)
                nc.vector.tensor_tensor(h13, spk3[:, :, 0:HF],
                                        spk3[:, :, HF:DN], Alu.add)
                h2 = sp.tile([TOK, HF], F16, tag="h2")
                h23 = h2[:].rearrange("p (t x) -> p t x", t=2)
                nc.vector.tensor_tensor(h23, h13[:, :, 0:HF // 2],
                                        h13[:, :, HF // 2:HF], Alu.add)
                nc.vector.tensor_tensor(
                    cnt4[:, tl * D:(tl + 2) * D].rearrange(
                        "p (t x) -> p t x", t=2),
                    h23[:, :, 0:D], h23[:, :, D:2 * D], Alu.add)

            if tl == TC - 1:
                # finish combine: s2 += phase, then out_u8 = 8*s2 + cnt
                ph = sp.tile([TOK, TC * D], F16, tag="ph")
                nc.vector.tensor_scalar(ph[:], wv4[:], 0.5, None, Alu.is_gt)
                s2 = s2s[c]
                nc.vector.tensor_tensor(s2[:], s2[:], ph[:], Alu.add)
                outst = lp.tile([TOK, TC * D], mybir.dt.uint8, tag="outst")
                nc.vector.scalar_tensor_tensor(outst[:], s2[:], 8.0,
                                               cnt4[:], Alu.mult, Alu.add)
                for j in range(TC):
                    nc.sync.dma_start(outd[t - TC + 1 + j],
                                      outst[:, j * D:(j + 1) * D])

    nc.compile()
    return nc


def _prepare_inputs(embeddings, pop_W, pop_b, freq_bands, enc_weights,
                    rate_noise, rate_rand, pop_rand):
    e = np.exp(enc_weights.astype(np.float64)
               - enc_weights.astype(np.float64).max())
    w = (e / e.sum()).astype(np.float32)
    w0, w1, w2, w3 = [float(x) for x in w]

    has_bias = bool(np.any(pop_b != 0))
    kdim = D + (128 if has_bias else 0)

    emb_f = np.ascontiguousarray(embeddings.reshape(NTOK, D))
    noise_f = np.ascontiguousarray(rate_noise.reshape(NTOK, D))
    # rate_rand [B,T,S,D] -> [BS, T, D] quantized to the 65535 grid
    rr_q = np.minimum(rate_rand.astype(np.float64) * 65535.0,
                      65535.0).astype(np.uint16)
    rr_f = np.ascontiguousarray(rr_q.transpose(0, 2, 1, 3).reshape(NTOK, T, D))
    # pop_rand [B,T,S,D,N] -> [BS, T, N*D] (n-major) in f16
    pr_f = np.ascontiguousarray(
        pop_rand.astype(np.float16).transpose(0, 2, 1, 4, 3)
        .reshape(NTOK, T, DN))
    # pop_W columns to n-major, f16
    W2 = np.ascontiguousarray(pop_W.reshape(D, D, N).transpose(0, 2, 1)
                              .reshape(D, DN))
    if has_bias:
        b_nm = np.ascontiguousarray(pop_b.reshape(D, N).T.reshape(1, DN))
        W2 = np.vstack([W2, b_nm, np.zeros((127, DN), np.float32)])
    import ml_dtypes
    W2 = np.ascontiguousarray(W2.astype(ml_dtypes.bfloat16))

    # wave rows: m[t,d] = (f32(t*f) mod 2pi centered to (-pi,pi]) - pi,
    # f16 hi/lo split.  matches jnp.linspace bit-exactly.
    import jax
    import jax.numpy as jnp
    with jax.default_device(jax.devices("cpu")[0]):
        t_lin = np.asarray(jnp.linspace(0.0, TWO_PI, T)).astype(np.float64)
    tfc = (t_lin[:, None] * freq_bands.astype(np.float64)[None, :]
           ).astype(np.float32).astype(np.float64)      # f32(t*f) as jax makes
    k0 = np.round(tfc / (2.0 * np.pi))
    m = tfc - k0 * (2.0 * np.pi)                        # in (-pi, pi]
    if USE_EXT_SIN:
        m = m - np.pi                                   # in (-2pi, 0]
    import ml_dtypes
    m_hi = m.astype(ml_dtypes.bfloat16)
    m_lo = (m - m_hi.astype(np.float64)).astype(ml_dtypes.bfloat16)
    tf = np.stack([m_hi.reshape(-1), m_lo.reshape(-1)])
    ident = np.eye(128, dtype=ml_dtypes.bfloat16)
    c_hi = 6.28125
    c_lo = 2.0 * np.pi - c_hi
    diag2 = np.hstack([np.eye(128) * (-c_hi),
                       np.eye(128) * (-c_lo)]).astype(np.float16)

    import ml_dtypes
    in_maps = []
    for c in range(NCORES):
        t0, t1 = c * TOK, (c + 1) * TOK
        embT = emb_f[t0:t1].T
        if has_bias:
            embT = np.vstack([embT, np.ones((1, TOK), np.float32),
                              np.zeros((127, TOK), np.float32)])
        in_maps.append({
            "emb": emb_f[t0:t1],
            "embT": np.ascontiguousarray(embT.astype(ml_dtypes.bfloat16)),
            "noise": noise_f[t0:t1],
            "rr": np.ascontiguousarray(rr_f[t0:t1].transpose(1, 0, 2)),
            "pr": np.ascontiguousarray(pr_f[t0:t1].transpose(1, 0, 2)),
            "W": W2,
            "tf": tf,
            "ident": ident,
            "diag2": diag2,
        })
    return in_maps, (w0, w1, w2, w3), has_bias


_cache = {}


def kernel(embeddings, pop_W, pop_b, freq_bands, enc_weights,
           rate_noise, rate_rand, pop_rand, _want_trace=False):
    in_maps, (w0, w1, w2, w3), has_bias = _prepare_inputs(
        embeddings, pop_W, pop_b, freq_bands, enc_weights,
        rate_noise, rate_rand, pop_rand)

    key = (w0, w1, w2, w3, has_bias)
    if key not in _cache:
        _cache[key] = _build_program(w0, w1, w2, w3, has_bias)
    nc = _cache[key]

    res = run_bass_kernel_spmd(nc, in_maps, core_ids=list(range(NCORES)),
                               trace=_want_trace)

    # out per core: [T, TOK, D] u8 holding 8*(r+temp+ph)+cnt -> f32 v/32
    full = np.empty((NTOK, T, D), np.float32)
    for c in range(NCORES):
        v = np.asarray(res.results[c]["out"]).astype(np.float32)
        full[c * TOK:(c + 1) * TOK] = v.transpose(1, 0, 2)
    out = full * (1.0 / 32.0)
    out = out.reshape(B, S, T, D).transpose(0, 2, 1, 3)
    out = np.ascontiguousarray(out.astype(np.float32))
    if _want_trace:
        kernel._last_trace = res
    return out
